# revision 1
# baseline (speedup 1.0000x reference)
"""GCN layer (message passing) on 8 Trainium2 NeuronCores via Bass/Tile — lean instruction count + bf16.

  m = (h @ W) * norm ; n = segment_sum(m[src], dst) ; out = leaky(n*norm+b)

Vs v1: host pre-scales h2 = h*norm (bf16), so the per-edge message is just
h2[src]. One-hot = pure is_equal built by ONE DVE op per group (broadcast
APs). Metadata (gather idx + local dst offsets) merged into one i16 DMA per
group. Matmuls in bf16. Epilogue: DVE mult(norm_dst) + fused Act Lrelu+bias.
"""

import sys

if "/opt/trn_rl_repo" not in sys.path:
    sys.path.insert(0, "/opt/trn_rl_repo")

import numpy as np
import ml_dtypes

import concourse.bass as bass
import concourse.bacc as bacc
import concourse.mybir as mybir
import concourse.tile as tile
from concourse.bass_utils import run_bass_kernel_spmd

P = 128
N = 100000
E = 1600000
D = 128
NCORES = 8
NPC = N // NCORES  # 12500 dst nodes per core
GN = 256  # dst nodes per group
G = (NPC + GN - 1) // GN  # 49
NBUCK = 4
BUCK = (N + NBUCK - 1) // NBUCK  # 25000 (int16-safe gather window)

f32 = mybir.dt.float32
i16 = mybir.dt.int16
bf16 = mybir.dt.bfloat16


def build_host_data(h, norm, weight, bias, src, dst):
    norm1 = np.ascontiguousarray(norm, dtype=np.float32).reshape(-1)
    h2 = (np.asarray(h, np.float32) * norm1[:, None]).astype(ml_dtypes.bfloat16)

    owner = dst // NPC
    cores = []
    counts_all = np.zeros((NCORES, G * NBUCK), np.int64)
    for c in range(NCORES):
        sel = owner == c
        src_c = src[sel]
        dst_c = dst[sel]
        ldst = dst_c - c * NPC
        key = (ldst // GN) * NBUCK + src_c // BUCK
        order = np.lexsort((src_c, key))
        key_s = key[order]
        counts = np.bincount(key_s, minlength=G * NBUCK)
        starts = np.zeros(G * NBUCK + 1, np.int64)
        np.cumsum(counts, out=starts[1:])
        rank = np.arange(len(key_s)) - starts[key_s]
        cores.append((src_c[order], dst_c[order], key_s, rank))
        counts_all[c] = counts

    s_gb = (counts_all.max(axis=0).reshape(G, NBUCK) + 127) // 128
    s_gb = np.maximum(s_gb, 1)
    slot_off = np.zeros((G, NBUCK), np.int64)
    for g in range(G):
        slot_off[g] = np.cumsum(np.concatenate([[0], s_gb[g][:-1]]))
    s_g = s_gb.sum(axis=1)
    SMAX = int(s_g.max())
    # idx columns: nidx/16 per bucket
    icols_gb = s_gb * 128 // 16
    icol_off = np.zeros((G, NBUCK), np.int64)
    for g in range(G):
        icol_off[g] = np.cumsum(np.concatenate([[0], icols_gb[g][:-1]]))
    ic_g = icols_gb.sum(axis=1)
    ICMAX = int(ic_g.max())
    MW = ICMAX + SMAX  # meta width: [idx i16 | lofs bf16-as-i16]

    iota = np.tile(
        np.arange(GN, dtype=np.float32).astype(ml_dtypes.bfloat16)[None, :], (P, 1)
    )
    w_bf = np.ascontiguousarray(weight).astype(ml_dtypes.bfloat16)
    bias_col = np.ascontiguousarray(bias).reshape(D, 1).astype(np.float32)

    in_maps = []
    for c in range(NCORES):
        src_s, dst_s, key_s, rank = cores[c]
        g_s = key_s // NBUCK
        b_s = key_s % NBUCK
        part = rank % 128
        slot = slot_off[g_s, b_s] + rank // 128

        # lofs: local dst offset within group, bf16; pad slots -> -1
        lofs = np.full((G, P, SMAX), -1.0, np.float32)
        lofs[g_s, part, slot] = (dst_s - c * NPC - g_s * GN).astype(np.float32)
        lofs_bf = lofs.astype(ml_dtypes.bfloat16)

        # gather idx: wrapped 16, pad -> 0 (gathers a real row; masked by lofs)
        idxw = np.zeros((G, 16, ICMAX), np.int16)
        loc = (src_s - b_s * BUCK).astype(np.int16)
        col = icol_off[g_s, b_s] * 16 + rank
        idxw[g_s, col % 16, col // 16] = loc
        idxw_full = np.broadcast_to(idxw[:, None, :, :], (G, 8, 16, ICMAX)).reshape(
            G, P, ICMAX
        )

        meta = np.zeros((G, P, MW), np.int16)
        meta[:, :, :ICMAX] = idxw_full
        meta[:, :, ICMAX:] = lofs_bf.view(np.int16)

        ngrp = np.zeros((G, GN), np.float32)
        nv = norm1[c * NPC : (c + 1) * NPC]
        ngrp.reshape(-1)[:NPC] = nv

        in_maps.append(
            {
                "h2": h2,
                "w": w_bf,
                "bias_col": bias_col,
                "iota": iota,
                "meta": np.ascontiguousarray(meta),
                "ngrp": ngrp,
            }
        )

    meta_d = {
        "s_gb": s_gb,
        "slot_off": slot_off,
        "s_g": s_g,
        "SMAX": SMAX,
        "icols_gb": icols_gb,
        "icol_off": icol_off,
        "ICMAX": ICMAX,
        "MW": MW,
    }
    return in_maps, meta_d


def build_program(
    meta,
    repeats: int = 1,
    variant: str = "full",
    hw_loop: bool = False,
    inner: int = 1,
    gath_bufs: int = 4,
):
    s_gb = meta["s_gb"]
    slot_off = meta["slot_off"]
    s_g = meta["s_g"]
    SMAX = meta["SMAX"]
    icols_gb = meta["icols_gb"]
    icol_off = meta["icol_off"]
    ICMAX = meta["ICMAX"]
    MW = meta["MW"]

    nc = bacc.Bacc(
        "TRN2",
        target_bir_lowering=False,
        debug=False,
        num_devices=NCORES,
        num_swdge_queues=4,
    )
    h2_d = nc.dram_tensor("h2", [N, D], bf16, kind="ExternalInput").ap()
    w_d = nc.dram_tensor("w", [D, D], bf16, kind="ExternalInput").ap()
    bias_d = nc.dram_tensor("bias_col", [D, 1], f32, kind="ExternalInput").ap()
    iota_d = nc.dram_tensor("iota", [P, GN], bf16, kind="ExternalInput").ap()
    meta_in = nc.dram_tensor("meta", [G, P, MW], i16, kind="ExternalInput").ap()
    ngrp_d = nc.dram_tensor("ngrp", [G, GN], f32, kind="ExternalInput").ap()
    outT_d = nc.dram_tensor("outT", [D, G * GN], f32, kind="ExternalOutput").ap()

    with tile.TileContext(nc) as tc:
        with (
            tc.tile_pool(name="consts", bufs=1) as consts,
            tc.tile_pool(name="meta_p", bufs=3) as meta_p,
            tc.tile_pool(name="gath", bufs=gath_bufs) as gath,
            tc.tile_pool(name="oh_p", bufs=3) as oh_p,
            tc.tile_pool(name="ep", bufs=3) as ep,
            tc.tile_pool(name="psum", bufs=4, space="PSUM") as psum,
        ):
            w_sb = consts.tile([P, D], bf16)
            nc.sync.dma_start(out=w_sb[:], in_=w_d[:, :])
            bias_sb = consts.tile([P, 1], f32)
            nc.sync.dma_start(out=bias_sb[:], in_=bias_d[:, :])
            iota_sb = consts.tile([P, GN], bf16)
            nc.sync.dma_start(out=iota_sb[:], in_=iota_d[:, :])

            def emit_group(g):
                SG = int(s_g[g])
                ICG = int(icols_gb[g].sum())
                meta_t = meta_p.tile([P, MW], i16, tag="meta")
                nc.sync.dma_start(
                    out=meta_t[:, : ICMAX + SG], in_=meta_in[g, :, : ICMAX + SG]
                )
                ngrp_t = meta_p.tile([P, GN], f32, tag="ngrp")
                ngrp_row = ngrp_d[g]
                ngrp_bc = bass.AP(
                    tensor=ngrp_row.tensor,
                    offset=ngrp_row.offset,
                    ap=[[0, P]] + list(ngrp_row.ap),
                )
                nc.sync.dma_start(out=ngrp_t[:], in_=ngrp_bc)

                hg = gath.tile([P, SMAX, D], bf16, tag="hg")
                if variant == "compute":
                    nc.sync.dma_start(
                        out=hg[:, :SG, :],
                        in_=h2_d[: SG * 128, :].rearrange(
                            "(s p) d -> p s d", p=P
                        ),
                    )
                else:
                    for b in range(NBUCK):
                        sb = int(s_gb[g, b])
                        nb = sb * 128
                        so = int(slot_off[g, b])
                        co = int(icol_off[g, b])
                        nc.gpsimd.dma_gather(
                            hg[:, so : so + sb, :],
                            h2_d[BUCK * b :, :],
                            meta_t[:, co : co + nb // 16],
                            nb,
                            nb,
                            D,
                            queue_num=b,
                            single_packet=(nb <= 1024),
                        )
                if variant == "gather":
                    t2 = ep.tile([P, GN], f32, tag="t2")
                    nc.vector.tensor_copy(
                        out=t2[:, :8], in_=hg[:, 0, :16].bitcast(f32)
                    )
                    nc.sync.dma_start(
                        out=outT_d[:, g * GN : (g + 1) * GN], in_=t2[:]
                    )
                    return

                # one-hot for the whole group in one DVE op:
                # oh[p, s*GN + j] = (iota[p, j] == lofs[p, s])
                oh = oh_p.tile([P, SMAX * GN], bf16, tag="oh")
                lofs_ap = meta_t[:, ICMAX : ICMAX + SG].bitcast(bf16)
                iota_b = iota_sb[:]
                in0 = bass.AP(
                    tensor=iota_b.tensor,
                    offset=iota_b.offset,
                    ap=[list(iota_b.ap[0]), [0, SG], [1, GN]],
                )
                in1 = bass.AP(
                    tensor=lofs_ap.tensor,
                    offset=lofs_ap.offset,
                    ap=[list(lofs_ap.ap[0]), [1, SG], [0, GN]],
                )
                nc.vector.tensor_tensor(
                    out=oh[:, : SG * GN],
                    in0=in0,
                    in1=in1,
                    op=mybir.AluOpType.is_equal,
                )

                ps_S = psum.tile([P, GN], f32, space="PSUM", tag="ps_S")
                for s in range(SG):
                    nc.tensor.matmul(
                        out=ps_S[:],
                        lhsT=hg[:, s, :],
                        rhs=oh[:, s * GN : (s + 1) * GN],
                        start=(s == 0),
                        stop=(s == SG - 1),
                    )

                s_sb = ep.tile([P, GN], bf16, tag="s_sb")
                nc.scalar.activation(
                    out=s_sb[:],
                    in_=ps_S[:],
                    func=mybir.ActivationFunctionType.Copy,
                )
                ps_O = psum.tile([P, GN], f32, space="PSUM", tag="ps_O")
                nc.tensor.matmul(
                    out=ps_O[:], lhsT=w_sb[:], rhs=s_sb[:], start=True, stop=True
                )
                t0 = ep.tile([P, GN], f32, tag="t0")
                nc.vector.tensor_tensor(
                    out=t0[:], in0=ps_O[:], in1=ngrp_t[:], op=mybir.AluOpType.mult
                )
                t1 = ep.tile([P, GN], f32, tag="t1")
                nc.scalar.activation(
                    out=t1[:],
                    in_=t0[:],
                    func=mybir.ActivationFunctionType.Identity,
                    bias=bias_sb[:, :1],
                )
                t2 = ep.tile([P, GN], f32, tag="t2")
                nc.vector.scalar_tensor_tensor(
                    out=t2[:],
                    in0=t1[:],
                    scalar=0.2,
                    in1=t1[:],
                    op0=mybir.AluOpType.mult,
                    op1=mybir.AluOpType.max,
                )
                nc.sync.dma_start(
                    out=outT_d[:, g * GN : (g + 1) * GN], in_=t2[:]
                )

            def emit_body():
                for g in range(G):
                    emit_group(g)

            if hw_loop and repeats > 1:
                with tc.For_i(0, repeats):
                    for _k in range(inner):
                        emit_body()
            else:
                for _rep in range(repeats):
                    emit_body()
    nc.compile()
    return nc


def run_program(nc, in_maps):
    res = run_bass_kernel_spmd(nc, in_maps, list(range(NCORES)))
    outs = []
    for c in range(NCORES):
        outT = res.results[c]["outT"]
        outs.append(outT[:, :NPC].T)
    return np.ascontiguousarray(np.concatenate(outs, axis=0))


def kernel(h, norm, weight, bias, src, dst):
    h = np.asarray(h, np.float32)
    norm = np.asarray(norm, np.float32)
    weight = np.asarray(weight, np.float32)
    bias = np.asarray(bias, np.float32)
    src = np.asarray(src, np.int32)
    dst = np.asarray(dst, np.int32)
    in_maps, meta = build_host_data(h, norm, weight, bias, src, dst)
    nc = build_program(meta)
    return run_program(nc, in_maps)

